# revision 3
# baseline (speedup 1.0000x reference)
"""Bass/TRN2 kernel v2.6 for binarized AlexNet-OWT-BN, 8-core data parallel.

Single pass per core over 1024 images, processed as two independent
512-image chunks (one fp32 PSUM bank each).  All convs are fp8 Toeplitz
matmuls in DoubleRow perf mode with taps paired (ky0,ky1) + (ky2, zero),
so one output-row chunk costs 512 PE cycles.  conv5 + the 7x7 mean fuse
into a 9-tap A-matrix (DR-paired).  Log-softmax is max-free (constant
shift folded into the head bias).

2x2 max-pooling of binary activations is an OR, computed as a bit-count:
the pre-pool threshold bits (evac1) feed a tiny DoubleRow matmul that sums
the 4 window bits (row pairs via the DR 2-dim, column parity via a
56-column summing matrix over the parity-major M layout), and a second
threshold (evac2, constant bias) re-binarizes.  This keeps pooling off the
critical DVE path and makes every inter-layer buffer fp8.

The PSUM evacuations are the serial bottleneck, so each image chunk evacs
on a different engine (per-layer knob): ACT Sign -> +-1 "tilde" domain,
DVE is_gt -> {0,1}; consumer weights / thresholds / pad constants are
folded per chunk domain on the host.  Chunk-0 matmuls, then its evac, then
chunk-1 matmuls are emitted in that order so the Tile scheduler cannot
chain one chunk's evac behind the other's.  Pad columns are eliminated
(Toeplitz rows dropped + per-column kappa); pad rows are Pool-engine
memsets; per-chunk PSUM tiles (2 banks, bufs=4) keep the conv/evac
pipeline 2 pairs deep.
"""

import os
import sys

sys.path.insert(0, "/opt/trn_rl_repo")

import numpy as np
import ml_dtypes

EPS = 1e-5
NCORE = 8
TP = 1024
NPER = TP
CH = 512
CSHIFT = 9.0

FP8 = ml_dtypes.float8_e4m3
BF16 = ml_dtypes.bfloat16

# evac1 engine per (layer, chunk): "act" -> tilde, "dve" -> {0,1}
EVC = {(1, 0): "act", (1, 1): "dve",
       (2, 0): "act", (2, 1): "dve",
       (3, 0): "act", (3, 1): "dve",
       (4, 0): "act", (4, 1): "dve"}
# evac2 (pool re-binarize) engine per (pool-layer, chunk)
EV2 = {(2, 0): "act", (2, 1): "dve",
       (4, 0): "act", (4, 1): "dve"}
for _k in list(EVC):
    _env = os.environ.get(f"KE{_k[0]}{_k[1]}")
    if _env:
        EVC[_k] = _env
for _k in list(EV2):
    _env = os.environ.get(f"KP{_k[0]}{_k[1]}")
    if _env:
        EV2[_k] = _env

LAYERS = {
    1: dict(ci=1, co=4, W=28, parity=False),
    2: dict(ci=4, co=4, W=28, parity=True),
    3: dict(ci=4, co=8, W=14, parity=False),
    4: dict(ci=8, co=8, W=14, parity=True),
}
KDIM = {1: 28, 2: 112, 3: 56, 4: 112}
SLOTS = {"xb": 31, "a2": 31, "a3": 17, "a4": 17, "a5": 10}
XDMA_SPLIT = (11, 11, 11, 11, 9, 9)  # slot-counts, interleaved c0/c1


def _mcol(ox, c, W, co, parity):
    if not parity:
        return ox * co + c
    half = (W // 2) * co
    pad_half = ((half + 31) // 32) * 32
    if ox % 2 == 0:
        return (ox // 2) * co + c
    return pad_half + (ox // 2) * co + c


def _mwidth(W, co, parity):
    if not parity:
        return W * co
    half = (W // 2) * co
    pad_half = ((half + 31) // 32) * 32
    return pad_half + half


MDIM = {l: _mwidth(g["W"], g["co"], g["parity"]) for l, g in LAYERS.items()}


def _toeplitz_real(wmat, W, parity):
    co, ci = wmat.shape[0], wmat.shape[1]
    K = W * ci
    M = _mwidth(W, co, parity)
    out = np.zeros((3, K, M), np.float64)
    colsum = np.zeros(M, np.float64)
    for ky in range(3):
        for ox in range(W):
            for kx in range(3):
                ix = ox + kx
                if not (1 <= ix <= W):
                    continue
                for c_o in range(co):
                    mc = _mcol(ox, c_o, W, co, parity)
                    for c_i in range(ci):
                        out[ky, (ix - 1) * ci + c_i, mc] = wmat[c_o, c_i, ky, kx]
                        colsum[mc] += wmat[c_o, c_i, ky, kx]
    return out, colsum


def _domains():
    """Input domain of each layer per chunk (layer 5 = head)."""
    dom = {(1, 0): "pm1", (1, 1): "pm1"}
    for c in (0, 1):
        dom[(2, c)] = "tilde" if EVC[(1, c)] == "act" else "01"
        dom[(3, c)] = "tilde" if EV2[(2, c)] == "act" else "01"
        dom[(4, c)] = "tilde" if EVC[(3, c)] == "act" else "01"
        dom[(5, c)] = "tilde" if EV2[(4, c)] == "act" else "01"
    return dom


def _fold_layer(inputs, l, dom_c):
    tag = str(l)
    w = np.asarray(inputs["w" + tag], np.float64)
    b = np.asarray(inputs["b" + tag], np.float64)
    g = np.asarray(inputs["g" + tag], np.float64)
    be = np.asarray(inputs["be" + tag], np.float64)
    m = np.asarray(inputs["m" + tag], np.float64)
    v = np.asarray(inputs["v" + tag], np.float64)
    wb = np.sign(w)
    s = g / np.sqrt(v + EPS)
    geo = LAYERS[l]
    co, W, parity = geo["co"], geo["W"], geo["parity"]
    halved = dom_c == "tilde"
    wmat = wb * 0.5 if halved else wb
    c = (b - m) + be / s
    flip = np.where(s < 0, -1.0, 1.0)
    wmat = wmat * flip[:, None, None, None]
    taps, colsum = _toeplitz_real(wmat, W, parity)
    kap_col = colsum if halved else np.zeros_like(colsum)
    M = MDIM[l]
    cb = np.zeros(M, np.float64)
    for ox in range(W):
        for c_o in range(co):
            cb[_mcol(ox, c_o, W, co, parity)] = (c * flip)[c_o]
    return taps, kap_col + cb


def _fold_head(inputs, dom_c):
    w5 = np.sign(np.asarray(inputs["w5"], np.float64))
    b5 = np.asarray(inputs["b5"], np.float64)
    g5 = np.asarray(inputs["g5"], np.float64)
    be5 = np.asarray(inputs["be5"], np.float64)
    m5 = np.asarray(inputs["m5"], np.float64)
    v5 = np.asarray(inputs["v5"], np.float64)
    s5 = g5 / np.sqrt(v5 + EPS)
    halved = dom_c == "tilde"
    w5f = w5 * 0.5 if halved else w5
    A = np.zeros((9, 56, 16), np.float64)
    for iy in range(9):
        for ix in range(1, 8):
            for ky in range(3):
                if not (0 <= iy - ky <= 6):
                    continue
                for kx in range(3):
                    if not (0 <= ix - kx <= 6):
                        continue
                    for ci in range(8):
                        A[iy, (ix - 1) * 8 + ci, :] += w5f[:, ci, ky, kx]
    kapA = A.sum(axis=(0, 1)) if halved else np.zeros(16)
    s5h = (s5 / 49.0).astype(np.float32)
    b5h = (s5 * (kapA / 49.0 + b5 - m5) + be5).astype(np.float32)
    return A, s5h, b5h


# fp8 blob layout: conv DR packs per (layer, chunk) + A-matrix packs +
# parity-sum matrices.  DoubleRow ldweights requires the tap-pair stride
# to be a multiple of 16 elements, so packs use Mpad = roundup(M, 16):
# entry -> (col offset, Mpad, M).
def _r16(m):
    return (m + 15) // 16 * 16


_OFF8 = {}
_o = 0


def _add8(key, m):
    global _o
    _OFF8[key] = (_o, _r16(m), m)
    _o += 2 * _r16(m)


for _l in (1, 2, 3, 4):
    for _c in (0, 1):
        _add8((f"{_l}c{_c}", "a"), MDIM[_l])
        _add8((f"{_l}c{_c}", "b"), MDIM[_l])
for _c in (0, 1):
    for _p in range(5):
        _add8((f"Ac{_c}", f"p{_p}"), 16)
for _l in (2, 4):
    _add8((f"P{_l}", "a"), 56)
NB8 = _o
_OFFB = {"hd": (0, 10)}
NBB = 10


def _host_fold(inputs):
    d = {}
    dom = _domains()

    def drpack(taps, mpad):
        K, M = taps.shape[1], taps.shape[2]
        pa = np.zeros((K, 2, mpad))
        pa[:, 0, :M], pa[:, 1, :M] = taps[0], taps[1]
        pb = np.zeros((K, 2, mpad))
        pb[:, 0, :M] = taps[2]
        return pa.reshape(K, -1), pb.reshape(K, -1)

    wf8 = np.zeros((128, NB8), np.float64)
    wbf = np.zeros((128, NBB), np.float64)
    thr = np.zeros((128, 26), np.float32)

    def put8(key, pack, arr):
        off, mpad, _m = _OFF8[(key, pack)]
        wf8[0:arr.shape[0], off:off + 2 * mpad] = arr

    def thrcol(i, vec, engine):
        v_ = -vec if engine == "dve" else vec
        thr[:len(v_), i] = v_.astype(np.float32)

    for l, base in ((1, 0), (2, 2), (3, 4), (4, 6)):
        for c in (0, 1):
            taps, t_ = _fold_layer(inputs, l, dom[(l, c)])
            thrcol(base + c, t_, EVC[(l, c)])
            mpad = _OFF8[(f"{l}c{c}", "a")][1]
            pa, pb = drpack(taps, mpad)
            put8(f"{l}c{c}", "a", pa)
            put8(f"{l}c{c}", "b", pb)

    for c in (0, 1):
        A, s5h, b5h = _fold_head(inputs, dom[(5, c)])
        At = np.concatenate([A, np.zeros((1, 56, 16))], axis=0)  # 10 taps
        for p in range(5):
            put8(f"Ac{c}", f"p{p}",
                 At[2 * p:2 * p + 2].transpose(1, 0, 2).reshape(56, 32))
        # Mpad == M == 16 for A packs, no padding needed
        thr[:16, 9 + c] = b5h
        if c == 0:
            thr[:16, 8] = s5h

    # parity-sum matrices [M_l, 2, 64pad]: evens at 0:56, odds at 64:120
    for l in (2, 4):
        mpad = _OFF8[(f"P{l}", "a")][1]
        P = np.zeros((MDIM[l], 2, mpad), np.float64)
        for m in range(56):
            P[m, :, m] = 1.0
            P[64 + m, :, m] = 1.0
        put8(f"P{l}", "a", P.reshape(MDIM[l], 2 * mpad))

    wl = np.sign(np.asarray(inputs["wl"], np.float64))
    bl = np.asarray(inputs["bl"], np.float64)
    off, width = _OFFB["hd"]
    wbf[0:16, off:off + width] = wl.T * 0.5
    thr[:10, 11] = (bl + 0.5 * wl.sum(axis=1) - CSHIFT).astype(np.float32)
    thr[:10, 12:22] = np.eye(10, dtype=np.float32)
    # evac2 thresholds: tilde bits -> s > -3.5, {0,1} bits -> s > 0.5;
    # ACT Sign uses +bias, DVE is_gt uses the raw threshold
    for i, (l, c) in enumerate(((2, 0), (2, 1), (4, 0), (4, 1))):
        tin = -3.5 if EVC[(l, c)] == "act" else 0.5
        thr[:56, 22 + i] = -tin if EV2[(l, c)] == "act" else tin

    d["wf8"] = wf8.astype(FP8)
    d["wbf"] = wbf.astype(BF16)
    d["thrblob"] = thr
    d["_padv"] = {(l, c): (-1.0 if dom[(l + 1, c)] == "tilde" else 0.0)
                  for l in (1, 2, 3, 4) for c in (0, 1)}
    return d


_CACHE = {}
KREPS = int(os.environ.get("KREPS", "1"))


def _build(padv=None):
    from concourse import bacc, tile, mybir

    dom = _domains()
    if padv is None:
        padv = {(l, c): (-1.0 if dom[(l + 1, c)] == "tilde" else 0.0)
                for l in (1, 2, 3, 4) for c in (0, 1)}

    f32 = mybir.dt.float32
    bf16 = mybir.dt.bfloat16
    fp8 = mybir.dt.float8e4
    ACT = mybir.ActivationFunctionType
    ALU = mybir.AluOpType
    AX = mybir.AxisListType
    DR = mybir.MatmulPerfMode.DoubleRow

    nc = bacc.Bacc("TRN2", num_devices=NCORE)

    xT = {c: nc.dram_tensor(f"xT{c}", (28, SLOTS["xb"] * CH), fp8,
                            kind="ExternalInput") for c in (0, 1)}
    wf8d = nc.dram_tensor("wf8", (128, NB8), fp8, kind="ExternalInput")
    wbfd = nc.dram_tensor("wbf", (128, NBB), bf16, kind="ExternalInput")
    thrd = nc.dram_tensor("thrblob", (128, 26), f32, kind="ExternalInput")
    out = nc.dram_tensor("out", (NPER, 10), f32, kind="ExternalOutput")

    with tile.TileContext(nc) as tc:
        stat = tc.alloc_tile_pool(name="stat", bufs=1)
        scr = tc.alloc_tile_pool(name="scr", bufs=3)
        tl = tc.alloc_tile_pool(name="tl", bufs=8)
        ps = tc.alloc_tile_pool(name="ps", bufs=4, space="PSUM")

        def cpair(name, p, slots, dt):
            return {c: stat.tile([p, slots * CH], dt, tag=f"{name}{c}",
                                 name=f"{name}{c}")
                    for c in (0, 1)}

        xb = cpair("xb", 28, SLOTS["xb"], fp8)
        a2 = cpair("a2", 112, SLOTS["a2"], fp8)
        a3 = cpair("a3", 56, SLOTS["a3"], fp8)
        a4 = cpair("a4", 112, SLOTS["a4"], fp8)
        a5 = cpair("a5", 56, SLOTS["a5"], fp8)

        wf8 = stat.tile([128, NB8], fp8, tag="wf8")
        nc.sync.dma_start(wf8[:, :], wf8d.ap())
        wbf = stat.tile([128, NBB], bf16, tag="wbf")
        nc.sync.dma_start(wbf[:, :], wbfd.ap())
        thr = stat.tile([128, 26], f32, tag="thr")
        nc.sync.dma_start(thr[:, :], thrd.ap())
        pos = {0: 0, 1: 0}
        for i, w_ in enumerate(XDMA_SPLIT):
            c = i % 2
            s0 = pos[c]
            nc.sync.dma_start(xb[c][:, s0 * CH:(s0 + w_) * CH],
                              xT[c].ap()[:, s0 * CH:(s0 + w_) * CH])
            pos[c] += w_
        for buf, np_, slots, l in ((a2, 112, (0, 29, 30), 1),
                                   (a3, 56, (0, 15, 16), 2),
                                   (a4, 112, (0, 15, 16), 3),
                                   (a5, 56, (0, 8, 9), 4)):
            for c in (0, 1):
                for s_ in slots:
                    nc.gpsimd.memset(buf[c][0:np_, s_ * CH:(s_ + 1) * CH],
                                     padv[(l, c)])

        def w8v(key, K):
            off, mpad, m = _OFF8[key]
            return wf8[0:K, off:off + 2 * mpad].rearrange(
                "k (h m) -> k h m", h=2)[0:K, :, 0:m]

        whd = wbf[0:16, _OFFB["hd"][0]:_OFFB["hd"][0] + 10]

        def rv(buf, name):
            return {c: buf[c][:, :].rearrange("k (r t) -> k r t",
                                              r=SLOTS[name]) for c in (0, 1)}

        xbv, a2v, a3v, a4v, a5v = (rv(xb, "xb"), rv(a2, "a2"), rv(a3, "a3"),
                                   rv(a4, "a4"), rv(a5, "a5"))

        def conv_chunk(l, srcv, k, c_, pt):
            """DR matmuls for rows (2k, 2k+1) of chunk c_ -> pt halves."""
            K, M = KDIM[l], MDIM[l]
            for h in (0, 1):
                y = 2 * k + h
                q = pt[0:M, h * CH:(h + 1) * CH]
                key = f"{l}c{c_}"
                nc.tensor.matmul(
                    out=q, lhsT=w8v((key, "a"), K),
                    rhs=srcv[c_][0:K, y:y + 2, :],
                    start=True, stop=False, perf_mode=DR, tile_position=(0, 0))
                nc.tensor.matmul(
                    out=q, lhsT=w8v((key, "b"), K),
                    rhs=srcv[c_][0:K, y + 2:y + 4, :],
                    start=False, stop=True, perf_mode=DR, tile_position=(0, 0))

        def evac_chunk(l, pt, c_, dst):
            """evac1: threshold pt [M, 2*CH] into fp8 dst."""
            M = MDIM[l]
            src = pt[0:M, 0:2 * CH]
            tcol = thr[0:M, 2 * (l - 1) + c_:2 * (l - 1) + c_ + 1]
            if EVC[(l, c_)] == "act":
                nc.scalar.activation(dst, src, ACT.Sign, bias=tcol)
            else:
                nc.vector.tensor_scalar(out=dst, in0=src,
                                        scalar1=tcol, scalar2=None,
                                        op0=ALU.is_gt)

        def dst_rows(buf, M, k):
            base = (2 * k + 1) * CH
            return [buf[c_][0:M, base:base + 2 * CH] for c_ in (0, 1)]

        def pool_bits(l, pt, c_):
            """evac1 the pre-pool bits into a held q0 tile."""
            M = MDIM[l]
            q0 = scr.tile([128, 2 * CH], fp8, tag=f"q0{c_}", name=f"q0{c_}",
                          bufs=3)
            evac_chunk(l, pt, c_, q0[0:M, :])
            return q0

        def pool_sum(l, q0, c_, dst, k):
            """Lagged bit-count matmul + evac2 into dst row k+1."""
            M = MDIM[l]
            sv = ps.tile([128, 1024], f32, tag="pt", name="sv")
            nc.tensor.matmul(
                out=sv[0:56, 0:CH], lhsT=w8v((f"P{l}", "a"), M),
                rhs=q0[0:M, :].rearrange("m (h t) -> m h t", h=2),
                start=True, stop=True, perf_mode=DR, tile_position=(0, 0))
            # s in {-4..4} (tilde bits) or {0..4} ({0,1} bits); pooled-OR
            ti = {(2, 0): 22, (2, 1): 23, (4, 0): 24, (4, 1): 25}[(l, c_)]
            tcol = thr[0:56, ti:ti + 1]
            dreg = dst[c_][0:56, (k + 1) * CH:(k + 2) * CH]
            if EV2[(l, c_)] == "act":
                nc.scalar.activation(dreg, sv[0:56, 0:CH], ACT.Sign,
                                     bias=tcol)
            else:
                nc.vector.tensor_scalar(out=dreg, in0=sv[0:56, 0:CH],
                                        scalar1=tcol, scalar2=None,
                                        op0=ALU.is_gt)

        srcs = {1: xbv, 2: a2v, 3: a3v, 4: a4v}
        dsts = {2: a3, 4: a5}

        def emit(l, k, pend):
            for c_ in (0, 1):
                pt = ps.tile([128, 1024], f32, tag="pt")
                conv_chunk(l, srcs[l], k, c_, pt)
                if l == 1:
                    evac_chunk(1, pt, c_, dst_rows(a2, 112, k)[c_])
                elif l == 3:
                    evac_chunk(3, pt, c_, dst_rows(a4, 112, k)[c_])
                else:
                    q0 = pool_bits(l, pt, c_)
                    if pend[c_] is not None:
                        pool_sum(l, pend[c_][0], c_, dsts[l], pend[c_][1])
                    pend[c_] = (q0, k)

        def flush(l, pend):
            for c_ in (0, 1):
                if pend[c_] is not None:
                    pool_sum(l, pend[c_][0], c_, dsts[l], pend[c_][1])
                    pend[c_] = None

        for rep in range(KREPS):
            pend = {0: None, 1: None}
            for k in range(14):
                emit(1, k, pend)
            for k in range(14):
                emit(2, k, pend)
            flush(2, pend)
            for k in range(7):
                emit(3, k, pend)
            for k in range(7):
                emit(4, k, pend)
            flush(4, pend)

            # ---- head: fused conv5 + mean, 10 taps as 5 DR packs ----
            u = scr.tile([16, TP], bf16, tag="u")
            hh = scr.tile([16, TP], f32, tag="hh")
            for c_ in (0, 1):
                pm = ps.tile([128, 1024], f32, tag="pt", name="pm")
                for p_ in range(5):
                    nc.tensor.matmul(
                        out=pm[0:16, 0:CH],
                        lhsT=w8v((f"Ac{c_}", f"p{p_}"), 56),
                        rhs=a5v[c_][0:56, 2 * p_:2 * p_ + 2, :],
                        start=(p_ == 0), stop=(p_ == 4),
                        perf_mode=DR, tile_position=(0, 0))
                nc.scalar.activation(u[0:16, c_ * CH:(c_ + 1) * CH],
                                     pm[0:16, 0:CH],
                                     ACT.Sign, bias=thr[0:16, 9 + c_:10 + c_],
                                     scale=thr[0:16, 8:9])
            for c_ in (0, 1):
                ph = ps.tile([128, 1024], f32, tag="pt", name="ph")
                nc.tensor.matmul(out=ph[0:10, 0:CH],
                                 lhsT=whd, rhs=u[0:16, c_ * CH:(c_ + 1) * CH],
                                 start=True, stop=True, tile_position=(0, 0))
                nc.scalar.activation(hh[0:10, c_ * CH:(c_ + 1) * CH],
                                     ph[0:10, 0:CH], ACT.Identity,
                                     bias=thr[0:10, 11:12])

            # ---- log-softmax tail ----
            hxs, exs = [], []
            for k in range(8):
                ptr = ps.tile([128, 1024], f32, tag="pt", name="ptr")
                nc.tensor.transpose(ptr[0:128, 0:10],
                                    hh[0:10, k * 128:(k + 1) * 128],
                                    thr[0:10, 12:22])
                hx = tl.tile([128, 16], f32, tag="hx")
                nc.vector.tensor_copy(hx[0:128, 0:10], ptr[0:128, 0:10])
                hxs.append(hx)
                ex = tl.tile([128, 16], f32, tag="ex")
                nc.scalar.activation(ex[0:128, 0:10], ptr[0:128, 0:10],
                                     ACT.Exp)
                exs.append(ex)
            sm = scr.tile([128, 8], f32, tag="sm")
            for k in range(8):
                nc.vector.tensor_reduce(sm[0:128, k:k + 1], exs[k][0:128, 0:10],
                                        axis=AX.X, op=ALU.add)
            lg = scr.tile([128, 8], f32, tag="lg")
            nc.scalar.activation(lg[0:128, 0:8], sm[0:128, 0:8], ACT.Ln)
            osb = scr.tile([128, 80], f32, tag="osb")
            for k in range(8):
                nc.vector.tensor_scalar(
                    out=osb[0:128, k * 10:(k + 1) * 10],
                    in0=hxs[k][0:128, 0:10], scalar1=lg[0:128, k:k + 1],
                    scalar2=None, op0=ALU.subtract)
            nc.sync.dma_start(
                out.ap().rearrange("(k r) c -> r k c", k=8), osb[:, :])

        for p in (ps, tl, scr, stat):
            p.release()

    nc.compile()
    return nc


def _prep_x(x):
    xs = np.sign(x.reshape(8192, 28, 28).astype(np.float32))
    arr = xs.transpose(2, 1, 0)
    blobs = []
    for co in range(NCORE):
        pair = []
        for c in (0, 1):
            b = np.zeros((28, SLOTS["xb"], CH), np.float32)
            i0 = co * TP + c * CH
            b[:, 1:29, :] = arr[:, :, i0:i0 + CH]
            pair.append(b.reshape(28, SLOTS["xb"] * CH).astype(FP8))
        blobs.append(pair)
    return blobs


def kernel(**inputs):
    from concourse.bass_utils import run_bass_kernel_spmd

    if "nc" not in _CACHE:
        _CACHE["nc"] = _build()
    nc = _CACHE["nc"]

    folded = _host_fold(inputs)
    xblobs = _prep_x(np.asarray(inputs["x"], np.float32))
    static = {k: folded[k] for k in ("wf8", "wbf", "thrblob")}

    in_maps = []
    for i in range(NCORE):
        m = {"xT0": xblobs[i][0], "xT1": xblobs[i][1]}
        m.update(static)
        in_maps.append(m)

    res = run_bass_kernel_spmd(nc, in_maps, core_ids=list(range(NCORE)))
    _CACHE["last_result"] = res
    outs = [res.results[i]["out"] for i in range(NCORE)]
    return np.concatenate(outs, axis=0).astype(np.float32)


# revision 4
# speedup vs baseline: 1.3498x; 1.3498x over previous
"""Bass/TRN2 kernel v2.6 for binarized AlexNet-OWT-BN, 8-core data parallel.

Single pass per core over 1024 images, processed as two independent
512-image chunks (one fp32 PSUM bank each).  All convs are fp8 Toeplitz
matmuls in DoubleRow perf mode with taps paired (ky0,ky1) + (ky2, zero),
so one output-row chunk costs 512 PE cycles.  conv5 + the 7x7 mean fuse
into a 9-tap A-matrix (DR-paired).  Log-softmax is max-free (constant
shift folded into the head bias).

2x2 max-pooling of binary activations is an OR, computed as a bit-count:
the pre-pool threshold bits (evac1) feed a tiny DoubleRow matmul that sums
the 4 window bits (row pairs via the DR 2-dim, column parity via a
56-column summing matrix over the parity-major M layout), and a second
threshold (evac2, constant bias) re-binarizes.  This keeps pooling off the
critical DVE path and makes every inter-layer buffer fp8.

The PSUM evacuations are the serial bottleneck, so each image chunk evacs
on a different engine (per-layer knob): ACT Sign -> +-1 "tilde" domain,
DVE is_gt -> {0,1}; consumer weights / thresholds / pad constants are
folded per chunk domain on the host.  Chunk-0 matmuls, then its evac, then
chunk-1 matmuls are emitted in that order so the Tile scheduler cannot
chain one chunk's evac behind the other's.  Pad columns are eliminated
(Toeplitz rows dropped + per-column kappa); pad rows are Pool-engine
memsets; per-chunk PSUM tiles (2 banks, bufs=4) keep the conv/evac
pipeline 2 pairs deep.
"""

import os
import sys

sys.path.insert(0, "/opt/trn_rl_repo")

import numpy as np
import ml_dtypes

EPS = 1e-5
NCORE = 8
TP = 1024
NPER = TP
CH = 512
CSHIFT = 9.0

FP8 = ml_dtypes.float8_e4m3
BF16 = ml_dtypes.bfloat16

# evac1 engine per (layer, chunk): "act" -> tilde, "dve" -> {0,1}
EVC = {(1, 0): "act", (1, 1): "dve",
       (2, 0): "act", (2, 1): "dve",
       (3, 0): "act", (3, 1): "dve",
       (4, 0): "act", (4, 1): "dve"}
# evac2 (pool re-binarize) engine per (pool-layer, chunk)
EV2 = {(2, 0): "act", (2, 1): "dve",
       (4, 0): "act", (4, 1): "dve"}
for _k in list(EVC):
    _env = os.environ.get(f"KE{_k[0]}{_k[1]}")
    if _env:
        EVC[_k] = _env
for _k in list(EV2):
    _env = os.environ.get(f"KP{_k[0]}{_k[1]}")
    if _env:
        EV2[_k] = _env

LAYERS = {
    1: dict(ci=1, co=4, W=28, parity=False),
    2: dict(ci=4, co=4, W=28, parity=True),
    3: dict(ci=4, co=8, W=14, parity=False),
    4: dict(ci=8, co=8, W=14, parity=True),
}
KDIM = {1: 28, 2: 112, 3: 56, 4: 112}
SLOTS = {"xb": 31, "a2": 31, "a3": 17, "a4": 17, "a5": 10}
XDMA_SPLIT = (5, 5, 9, 9, 9, 9, 8, 8)  # slot-counts, interleaved c0/c1


def _mcol(ox, c, W, co, parity):
    if not parity:
        return ox * co + c
    half = (W // 2) * co
    pad_half = ((half + 31) // 32) * 32
    if ox % 2 == 0:
        return (ox // 2) * co + c
    return pad_half + (ox // 2) * co + c


def _mwidth(W, co, parity):
    if not parity:
        return W * co
    half = (W // 2) * co
    pad_half = ((half + 31) // 32) * 32
    return pad_half + half


MDIM = {l: _mwidth(g["W"], g["co"], g["parity"]) for l, g in LAYERS.items()}


def _toeplitz_real(wmat, W, parity):
    co, ci = wmat.shape[0], wmat.shape[1]
    K = W * ci
    M = _mwidth(W, co, parity)
    out = np.zeros((3, K, M), np.float64)
    colsum = np.zeros(M, np.float64)
    for ky in range(3):
        for ox in range(W):
            for kx in range(3):
                ix = ox + kx
                if not (1 <= ix <= W):
                    continue
                for c_o in range(co):
                    mc = _mcol(ox, c_o, W, co, parity)
                    for c_i in range(ci):
                        out[ky, (ix - 1) * ci + c_i, mc] = wmat[c_o, c_i, ky, kx]
                        colsum[mc] += wmat[c_o, c_i, ky, kx]
    return out, colsum


def _domains():
    """Input domain of each layer per chunk (layer 5 = head)."""
    dom = {(1, 0): "pm1", (1, 1): "pm1"}
    for c in (0, 1):
        dom[(2, c)] = "tilde" if EVC[(1, c)] == "act" else "01"
        dom[(3, c)] = "tilde" if EV2[(2, c)] == "act" else "01"
        dom[(4, c)] = "tilde" if EVC[(3, c)] == "act" else "01"
        dom[(5, c)] = "tilde" if EV2[(4, c)] == "act" else "01"
    return dom


def _fold_layer(inputs, l, dom_c):
    tag = str(l)
    w = np.asarray(inputs["w" + tag], np.float64)
    b = np.asarray(inputs["b" + tag], np.float64)
    g = np.asarray(inputs["g" + tag], np.float64)
    be = np.asarray(inputs["be" + tag], np.float64)
    m = np.asarray(inputs["m" + tag], np.float64)
    v = np.asarray(inputs["v" + tag], np.float64)
    wb = np.sign(w)
    s = g / np.sqrt(v + EPS)
    geo = LAYERS[l]
    co, W, parity = geo["co"], geo["W"], geo["parity"]
    halved = dom_c == "tilde"
    wmat = wb * 0.5 if halved else wb
    c = (b - m) + be / s
    flip = np.where(s < 0, -1.0, 1.0)
    wmat = wmat * flip[:, None, None, None]
    taps, colsum = _toeplitz_real(wmat, W, parity)
    kap_col = colsum if halved else np.zeros_like(colsum)
    M = MDIM[l]
    cb = np.zeros(M, np.float64)
    for ox in range(W):
        for c_o in range(co):
            cb[_mcol(ox, c_o, W, co, parity)] = (c * flip)[c_o]
    return taps, kap_col + cb


def _fold_head(inputs, dom_c):
    w5 = np.sign(np.asarray(inputs["w5"], np.float64))
    b5 = np.asarray(inputs["b5"], np.float64)
    g5 = np.asarray(inputs["g5"], np.float64)
    be5 = np.asarray(inputs["be5"], np.float64)
    m5 = np.asarray(inputs["m5"], np.float64)
    v5 = np.asarray(inputs["v5"], np.float64)
    s5 = g5 / np.sqrt(v5 + EPS)
    halved = dom_c == "tilde"
    w5f = w5 * 0.5 if halved else w5
    A = np.zeros((9, 56, 16), np.float64)
    for iy in range(9):
        for ix in range(1, 8):
            for ky in range(3):
                if not (0 <= iy - ky <= 6):
                    continue
                for kx in range(3):
                    if not (0 <= ix - kx <= 6):
                        continue
                    for ci in range(8):
                        A[iy, (ix - 1) * 8 + ci, :] += w5f[:, ci, ky, kx]
    kapA = A.sum(axis=(0, 1)) if halved else np.zeros(16)
    s5h = (s5 / 49.0).astype(np.float32)
    b5h = (s5 * (kapA / 49.0 + b5 - m5) + be5).astype(np.float32)
    return A, s5h, b5h


# fp8 blob layout: conv DR packs per (layer, chunk) + A-matrix packs +
# parity-sum matrices.  DoubleRow ldweights requires the tap-pair stride
# to be a multiple of 16 elements, so packs use Mpad = roundup(M, 16):
# entry -> (col offset, Mpad, M).
def _r16(m):
    return (m + 15) // 16 * 16


_OFF8 = {}
_o = 0


def _add8(key, m):
    global _o
    _OFF8[key] = (_o, _r16(m), m)
    _o += 2 * _r16(m)


for _l in (1, 2, 3, 4):
    for _c in (0, 1):
        _add8((f"{_l}c{_c}", "a"), MDIM[_l])
        _add8((f"{_l}c{_c}", "b"), MDIM[_l])
for _c in (0, 1):
    for _p in range(5):
        _add8((f"Ac{_c}", f"p{_p}"), 16)
for _l in (2, 4):
    _add8((f"P{_l}", "a"), 56)
NB8 = _o
_OFFB = {"hd": (0, 10)}
NBB = 10


def _host_fold(inputs):
    d = {}
    dom = _domains()

    def drpack(taps, mpad):
        K, M = taps.shape[1], taps.shape[2]
        pa = np.zeros((K, 2, mpad))
        pa[:, 0, :M], pa[:, 1, :M] = taps[0], taps[1]
        pb = np.zeros((K, 2, mpad))
        pb[:, 0, :M] = taps[2]
        return pa.reshape(K, -1), pb.reshape(K, -1)

    wf8 = np.zeros((128, NB8), np.float64)
    wbf = np.zeros((128, NBB), np.float64)
    thr = np.zeros((128, 26), np.float32)

    def put8(key, pack, arr):
        off, mpad, _m = _OFF8[(key, pack)]
        wf8[0:arr.shape[0], off:off + 2 * mpad] = arr

    def thrcol(i, vec, engine):
        v_ = -vec if engine == "dve" else vec
        thr[:len(v_), i] = v_.astype(np.float32)

    for l, base in ((1, 0), (2, 2), (3, 4), (4, 6)):
        for c in (0, 1):
            taps, t_ = _fold_layer(inputs, l, dom[(l, c)])
            thrcol(base + c, t_, EVC[(l, c)])
            mpad = _OFF8[(f"{l}c{c}", "a")][1]
            pa, pb = drpack(taps, mpad)
            put8(f"{l}c{c}", "a", pa)
            put8(f"{l}c{c}", "b", pb)

    for c in (0, 1):
        A, s5h, b5h = _fold_head(inputs, dom[(5, c)])
        At = np.concatenate([A, np.zeros((1, 56, 16))], axis=0)  # 10 taps
        for p in range(5):
            put8(f"Ac{c}", f"p{p}",
                 At[2 * p:2 * p + 2].transpose(1, 0, 2).reshape(56, 32))
        # Mpad == M == 16 for A packs, no padding needed
        thr[:16, 9 + c] = b5h
        if c == 0:
            thr[:16, 8] = s5h

    # parity-sum matrices [M_l, 2, 64pad]: evens at 0:56, odds at 64:120
    for l in (2, 4):
        mpad = _OFF8[(f"P{l}", "a")][1]
        P = np.zeros((MDIM[l], 2, mpad), np.float64)
        for m in range(56):
            P[m, :, m] = 1.0
            P[64 + m, :, m] = 1.0
        put8(f"P{l}", "a", P.reshape(MDIM[l], 2 * mpad))

    wl = np.sign(np.asarray(inputs["wl"], np.float64))
    bl = np.asarray(inputs["bl"], np.float64)
    off, width = _OFFB["hd"]
    wbf[0:16, off:off + width] = wl.T * 0.5
    thr[:10, 11] = (bl + 0.5 * wl.sum(axis=1) - CSHIFT).astype(np.float32)
    thr[:10, 12:22] = np.eye(10, dtype=np.float32)
    # evac2 thresholds: tilde bits -> s > -3.5, {0,1} bits -> s > 0.5;
    # ACT Sign uses +bias, DVE is_gt uses the raw threshold
    for i, (l, c) in enumerate(((2, 0), (2, 1), (4, 0), (4, 1))):
        tin = -3.5 if EVC[(l, c)] == "act" else 0.5
        thr[:56, 22 + i] = -tin if EV2[(l, c)] == "act" else tin

    d["wf8"] = wf8.astype(FP8)
    d["wbf"] = wbf.astype(BF16)
    d["thrblob"] = thr
    d["_padv"] = {(l, c): (-1.0 if dom[(l + 1, c)] == "tilde" else 0.0)
                  for l in (1, 2, 3, 4) for c in (0, 1)}
    return d


_CACHE = {}
KREPS = int(os.environ.get("KREPS", "1"))


def _build(padv=None):
    from concourse import bacc, tile, mybir

    dom = _domains()
    if padv is None:
        padv = {(l, c): (-1.0 if dom[(l + 1, c)] == "tilde" else 0.0)
                for l in (1, 2, 3, 4) for c in (0, 1)}

    f32 = mybir.dt.float32
    bf16 = mybir.dt.bfloat16
    fp8 = mybir.dt.float8e4
    ACT = mybir.ActivationFunctionType
    ALU = mybir.AluOpType
    AX = mybir.AxisListType
    DR = mybir.MatmulPerfMode.DoubleRow

    nc = bacc.Bacc("TRN2", num_devices=NCORE)

    xT = {c: nc.dram_tensor(f"xT{c}", (28, SLOTS["xb"] * CH), fp8,
                            kind="ExternalInput") for c in (0, 1)}
    wf8d = nc.dram_tensor("wf8", (128, NB8), fp8, kind="ExternalInput")
    wbfd = nc.dram_tensor("wbf", (128, NBB), bf16, kind="ExternalInput")
    thrd = nc.dram_tensor("thrblob", (128, 26), f32, kind="ExternalInput")
    out = nc.dram_tensor("out", (NPER, 10), f32, kind="ExternalOutput")

    with tile.TileContext(nc) as tc:
        stat = tc.alloc_tile_pool(name="stat", bufs=1)
        scr = tc.alloc_tile_pool(name="scr", bufs=3)
        tl = tc.alloc_tile_pool(name="tl", bufs=8)
        ps = tc.alloc_tile_pool(name="ps", bufs=4, space="PSUM")

        def cpair(name, p, slots, dt):
            return {c: stat.tile([p, slots * CH], dt, tag=f"{name}{c}",
                                 name=f"{name}{c}")
                    for c in (0, 1)}

        xb = cpair("xb", 28, SLOTS["xb"], fp8)
        a2 = cpair("a2", 112, SLOTS["a2"], fp8)
        a3 = cpair("a3", 56, SLOTS["a3"], fp8)
        a4 = cpair("a4", 112, SLOTS["a4"], fp8)
        a5 = cpair("a5", 56, SLOTS["a5"], fp8)

        wf8 = stat.tile([128, NB8], fp8, tag="wf8")
        wbf = stat.tile([128, NBB], bf16, tag="wbf")
        thr = stat.tile([128, 26], f32, tag="thr")
        pos = {0: 0, 1: 0}

        def xdma(i):
            c = i % 2
            s0 = pos[c]
            w_ = XDMA_SPLIT[i]
            nc.sync.dma_start(xb[c][:, s0 * CH:(s0 + w_) * CH],
                              xT[c].ap()[:, s0 * CH:(s0 + w_) * CH])
            pos[c] += w_

        # first x slices + conv weights first: they gate L1 pair 0
        xdma(0)
        nc.sync.dma_start(wf8[:, :], wf8d.ap())
        xdma(1)
        nc.sync.dma_start(thr[:, :], thrd.ap())
        for i in range(2, len(XDMA_SPLIT)):
            xdma(i)
        nc.sync.dma_start(wbf[:, :], wbfd.ap())
        for buf, np_, slots, l in ((a2, 112, (0, 29, 30), 1),
                                   (a3, 56, (0, 15, 16), 2),
                                   (a4, 112, (0, 15, 16), 3),
                                   (a5, 56, (0, 8, 9), 4)):
            for c in (0, 1):
                for s_ in slots:
                    nc.gpsimd.memset(buf[c][0:np_, s_ * CH:(s_ + 1) * CH],
                                     padv[(l, c)])

        def w8v(key, K):
            off, mpad, m = _OFF8[key]
            return wf8[0:K, off:off + 2 * mpad].rearrange(
                "k (h m) -> k h m", h=2)[0:K, :, 0:m]

        whd = wbf[0:16, _OFFB["hd"][0]:_OFFB["hd"][0] + 10]

        def rv(buf, name):
            return {c: buf[c][:, :].rearrange("k (r t) -> k r t",
                                              r=SLOTS[name]) for c in (0, 1)}

        xbv, a2v, a3v, a4v, a5v = (rv(xb, "xb"), rv(a2, "a2"), rv(a3, "a3"),
                                   rv(a4, "a4"), rv(a5, "a5"))

        def conv_chunk(l, srcv, k, c_, pt):
            """DR matmuls for rows (2k, 2k+1) of chunk c_ -> pt halves."""
            K, M = KDIM[l], MDIM[l]
            for h in (0, 1):
                y = 2 * k + h
                q = pt[0:M, h * CH:(h + 1) * CH]
                key = f"{l}c{c_}"
                nc.tensor.matmul(
                    out=q, lhsT=w8v((key, "a"), K),
                    rhs=srcv[c_][0:K, y:y + 2, :],
                    start=True, stop=False, perf_mode=DR, tile_position=(0, 0))
                nc.tensor.matmul(
                    out=q, lhsT=w8v((key, "b"), K),
                    rhs=srcv[c_][0:K, y + 2:y + 4, :],
                    start=False, stop=True, perf_mode=DR, tile_position=(0, 0))

        def evac_chunk(l, pt, c_, dst):
            """evac1: threshold pt [M, 2*CH] into fp8 dst."""
            M = MDIM[l]
            src = pt[0:M, 0:2 * CH]
            tcol = thr[0:M, 2 * (l - 1) + c_:2 * (l - 1) + c_ + 1]
            if EVC[(l, c_)] == "act":
                nc.scalar.activation(dst, src, ACT.Sign, bias=tcol)
            else:
                nc.vector.tensor_scalar(out=dst, in0=src,
                                        scalar1=tcol, scalar2=None,
                                        op0=ALU.is_gt)

        def dst_rows(buf, M, k):
            base = (2 * k + 1) * CH
            return [buf[c_][0:M, base:base + 2 * CH] for c_ in (0, 1)]

        def pool_bits(l, pt, c_):
            """evac1 the pre-pool bits into a held q0 tile."""
            M = MDIM[l]
            q0 = scr.tile([128, 2 * CH], fp8, tag=f"q0{c_}", name=f"q0{c_}",
                          bufs=3)
            evac_chunk(l, pt, c_, q0[0:M, :])
            return q0

        def pool_sum(l, q0, c_, dst, k):
            """Lagged bit-count matmul + evac2 into dst row k+1."""
            M = MDIM[l]
            sv = ps.tile([128, 1024], f32, tag="pt", name="sv")
            nc.tensor.matmul(
                out=sv[0:56, 0:CH], lhsT=w8v((f"P{l}", "a"), M),
                rhs=q0[0:M, :].rearrange("m (h t) -> m h t", h=2),
                start=True, stop=True, perf_mode=DR, tile_position=(0, 0))
            # s in {-4..4} (tilde bits) or {0..4} ({0,1} bits); pooled-OR
            ti = {(2, 0): 22, (2, 1): 23, (4, 0): 24, (4, 1): 25}[(l, c_)]
            tcol = thr[0:56, ti:ti + 1]
            dreg = dst[c_][0:56, (k + 1) * CH:(k + 2) * CH]
            if EV2[(l, c_)] == "act":
                nc.scalar.activation(dreg, sv[0:56, 0:CH], ACT.Sign,
                                     bias=tcol)
            else:
                nc.vector.tensor_scalar(out=dreg, in0=sv[0:56, 0:CH],
                                        scalar1=tcol, scalar2=None,
                                        op0=ALU.is_gt)

        srcs = {1: xbv, 2: a2v, 3: a3v, 4: a4v}
        dsts = {2: a3, 4: a5}

        def emit(l, k, pend):
            for c_ in (0, 1):
                pt = ps.tile([128, 1024], f32, tag="pt")
                conv_chunk(l, srcs[l], k, c_, pt)
                if l == 1:
                    evac_chunk(1, pt, c_, dst_rows(a2, 112, k)[c_])
                elif l == 3:
                    evac_chunk(3, pt, c_, dst_rows(a4, 112, k)[c_])
                else:
                    q0 = pool_bits(l, pt, c_)
                    if pend[c_] is not None:
                        pool_sum(l, pend[c_][0], c_, dsts[l], pend[c_][1])
                    pend[c_] = (q0, k)

        def flush(l, pend):
            for c_ in (0, 1):
                if pend[c_] is not None:
                    pool_sum(l, pend[c_][0], c_, dsts[l], pend[c_][1])
                    pend[c_] = None

        for rep in range(KREPS):
            pend = {0: None, 1: None}
            for k in range(14):
                emit(1, k, pend)
            for k in range(14):
                emit(2, k, pend)
            flush(2, pend)
            for k in range(7):
                emit(3, k, pend)
            for k in range(7):
                emit(4, k, pend)
            flush(4, pend)

            # ---- head: fused conv5 + mean, 10 taps as 5 DR packs ----
            u = scr.tile([16, TP], bf16, tag="u")
            hh = scr.tile([16, TP], f32, tag="hh")
            for c_ in (0, 1):
                pm = ps.tile([128, 1024], f32, tag="pt", name="pm")
                for p_ in range(5):
                    nc.tensor.matmul(
                        out=pm[0:16, 0:CH],
                        lhsT=w8v((f"Ac{c_}", f"p{p_}"), 56),
                        rhs=a5v[c_][0:56, 2 * p_:2 * p_ + 2, :],
                        start=(p_ == 0), stop=(p_ == 4),
                        perf_mode=DR, tile_position=(0, 0))
                nc.scalar.activation(u[0:16, c_ * CH:(c_ + 1) * CH],
                                     pm[0:16, 0:CH],
                                     ACT.Sign, bias=thr[0:16, 9 + c_:10 + c_],
                                     scale=thr[0:16, 8:9])
            for c_ in (0, 1):
                ph = ps.tile([128, 1024], f32, tag="pt", name="ph")
                nc.tensor.matmul(out=ph[0:10, 0:CH],
                                 lhsT=whd, rhs=u[0:16, c_ * CH:(c_ + 1) * CH],
                                 start=True, stop=True, tile_position=(0, 0))
                nc.scalar.activation(hh[0:10, c_ * CH:(c_ + 1) * CH],
                                     ph[0:10, 0:CH], ACT.Identity,
                                     bias=thr[0:10, 11:12])

            # ---- log-softmax tail ----
            hxs, exs = [], []
            for k in range(8):
                ptr = ps.tile([128, 1024], f32, tag="pt", name="ptr")
                nc.tensor.transpose(ptr[0:128, 0:10],
                                    hh[0:10, k * 128:(k + 1) * 128],
                                    thr[0:10, 12:22])
                hx = tl.tile([128, 16], f32, tag="hx")
                nc.vector.tensor_copy(hx[0:128, 0:10], ptr[0:128, 0:10])
                hxs.append(hx)
                ex = tl.tile([128, 16], f32, tag="ex")
                nc.scalar.activation(ex[0:128, 0:10], ptr[0:128, 0:10],
                                     ACT.Exp)
                exs.append(ex)
            sm = scr.tile([128, 8], f32, tag="sm")
            for k in range(8):
                nc.vector.tensor_reduce(sm[0:128, k:k + 1], exs[k][0:128, 0:10],
                                        axis=AX.X, op=ALU.add)
            lg = scr.tile([128, 8], f32, tag="lg")
            nc.scalar.activation(lg[0:128, 0:8], sm[0:128, 0:8], ACT.Ln)
            osb = scr.tile([128, 80], f32, tag="osb")
            for k in range(8):
                nc.vector.tensor_scalar(
                    out=osb[0:128, k * 10:(k + 1) * 10],
                    in0=hxs[k][0:128, 0:10], scalar1=lg[0:128, k:k + 1],
                    scalar2=None, op0=ALU.subtract)
            nc.sync.dma_start(
                out.ap().rearrange("(k r) c -> r k c", k=8), osb[:, :])

        for p in (ps, tl, scr, stat):
            p.release()

    nc.compile()
    return nc


def _prep_x(x):
    xs = np.sign(x.reshape(8192, 28, 28).astype(np.float32))
    arr = xs.transpose(2, 1, 0)
    blobs = []
    for co in range(NCORE):
        pair = []
        for c in (0, 1):
            b = np.zeros((28, SLOTS["xb"], CH), np.float32)
            i0 = co * TP + c * CH
            b[:, 1:29, :] = arr[:, :, i0:i0 + CH]
            pair.append(b.reshape(28, SLOTS["xb"] * CH).astype(FP8))
        blobs.append(pair)
    return blobs


def kernel(**inputs):
    from concourse.bass_utils import run_bass_kernel_spmd

    if "nc" not in _CACHE:
        _CACHE["nc"] = _build()
    nc = _CACHE["nc"]

    folded = _host_fold(inputs)
    xblobs = _prep_x(np.asarray(inputs["x"], np.float32))
    static = {k: folded[k] for k in ("wf8", "wbf", "thrblob")}

    in_maps = []
    for i in range(NCORE):
        m = {"xT0": xblobs[i][0], "xT1": xblobs[i][1]}
        m.update(static)
        in_maps.append(m)

    res = run_bass_kernel_spmd(nc, in_maps, core_ids=list(range(NCORE)))
    _CACHE["last_result"] = res
    outs = [res.results[i]["out"] for i in range(NCORE)]
    return np.concatenate(outs, axis=0).astype(np.float32)


# revision 5
# speedup vs baseline: 2.0169x; 1.4943x over previous
"""Bass/TRN2 kernel v2.6 for binarized AlexNet-OWT-BN, 8-core data parallel.

Single pass per core over 1024 images, processed as two independent
512-image chunks (one fp32 PSUM bank each).  All convs are fp8 Toeplitz
matmuls in DoubleRow perf mode with taps paired (ky0,ky1) + (ky2, zero),
so one output-row chunk costs 512 PE cycles.  conv5 + the 7x7 mean fuse
into a 9-tap A-matrix (DR-paired).  Log-softmax is max-free (constant
shift folded into the head bias).

2x2 max-pooling of binary activations is an OR, computed as a bit-count:
the pre-pool threshold bits (evac1) feed a tiny DoubleRow matmul that sums
the 4 window bits (row pairs via the DR 2-dim, column parity via a
56-column summing matrix over the parity-major M layout), and a second
threshold (evac2, constant bias) re-binarizes.  This keeps pooling off the
critical DVE path and makes every inter-layer buffer fp8.

The PSUM evacuations are the serial bottleneck, so each image chunk evacs
on a different engine (per-layer knob): ACT Sign -> +-1 "tilde" domain,
DVE is_gt -> {0,1}; consumer weights / thresholds / pad constants are
folded per chunk domain on the host.  Chunk-0 matmuls, then its evac, then
chunk-1 matmuls are emitted in that order so the Tile scheduler cannot
chain one chunk's evac behind the other's.  Pad columns are eliminated
(Toeplitz rows dropped + per-column kappa); pad rows are Pool-engine
memsets; per-chunk PSUM tiles (2 banks, bufs=4) keep the conv/evac
pipeline 2 pairs deep.
"""

import os
import sys

sys.path.insert(0, "/opt/trn_rl_repo")

import numpy as np
import ml_dtypes

EPS = 1e-5
NCORE = 8
TP = 1024
NPER = TP
CH = 512
CSHIFT = 9.0

FP8 = ml_dtypes.float8_e4m3
BF16 = ml_dtypes.bfloat16

# evac1 engine per (layer, chunk): "act" -> tilde, "dve" -> {0,1}
EVC = {(1, 0): "act", (1, 1): "dve",
       (2, 0): "act", (2, 1): "dve",
       (3, 0): "act", (3, 1): "dve",
       (4, 0): "act", (4, 1): "dve"}
# evac2 (pool re-binarize) engine per (pool-layer, chunk)
EV2 = {(2, 0): "act", (2, 1): "dve",
       (4, 0): "act", (4, 1): "dve"}
for _k in list(EVC):
    _env = os.environ.get(f"KE{_k[0]}{_k[1]}")
    if _env:
        EVC[_k] = _env
for _k in list(EV2):
    _env = os.environ.get(f"KP{_k[0]}{_k[1]}")
    if _env:
        EV2[_k] = _env

LAYERS = {
    1: dict(ci=1, co=4, W=28, parity=False),
    2: dict(ci=4, co=4, W=28, parity=True),
    3: dict(ci=4, co=8, W=14, parity=False),
    4: dict(ci=8, co=8, W=14, parity=True),
}
KDIM = {1: 28, 2: 112, 3: 56, 4: 112}
SLOTS = {"xb": 31, "a2": 31, "a3": 17, "a4": 17, "a5": 10}
XDMA_SPLIT = (5, 5, 9, 9, 9, 9, 8, 8)  # slot-counts, interleaved c0/c1


def _mcol(ox, c, W, co, parity):
    if not parity:
        return ox * co + c
    half = (W // 2) * co
    pad_half = ((half + 31) // 32) * 32
    if ox % 2 == 0:
        return (ox // 2) * co + c
    return pad_half + (ox // 2) * co + c


def _mwidth(W, co, parity):
    if not parity:
        return W * co
    half = (W // 2) * co
    pad_half = ((half + 31) // 32) * 32
    return pad_half + half


MDIM = {l: _mwidth(g["W"], g["co"], g["parity"]) for l, g in LAYERS.items()}


def _toeplitz_real(wmat, W, parity):
    co, ci = wmat.shape[0], wmat.shape[1]
    K = W * ci
    M = _mwidth(W, co, parity)
    out = np.zeros((3, K, M), np.float64)
    colsum = np.zeros(M, np.float64)
    for ky in range(3):
        for ox in range(W):
            for kx in range(3):
                ix = ox + kx
                if not (1 <= ix <= W):
                    continue
                for c_o in range(co):
                    mc = _mcol(ox, c_o, W, co, parity)
                    for c_i in range(ci):
                        out[ky, (ix - 1) * ci + c_i, mc] = wmat[c_o, c_i, ky, kx]
                        colsum[mc] += wmat[c_o, c_i, ky, kx]
    return out, colsum


def _domains():
    """Input domain of each layer per chunk (layer 5 = head)."""
    dom = {(1, 0): "pm1", (1, 1): "pm1"}
    for c in (0, 1):
        dom[(2, c)] = "tilde" if EVC[(1, c)] == "act" else "01"
        dom[(3, c)] = "tilde" if EV2[(2, c)] == "act" else "01"
        dom[(4, c)] = "tilde" if EVC[(3, c)] == "act" else "01"
        dom[(5, c)] = "tilde" if EV2[(4, c)] == "act" else "01"
    return dom


def _fold_layer(inputs, l, dom_c):
    tag = str(l)
    w = np.asarray(inputs["w" + tag], np.float64)
    b = np.asarray(inputs["b" + tag], np.float64)
    g = np.asarray(inputs["g" + tag], np.float64)
    be = np.asarray(inputs["be" + tag], np.float64)
    m = np.asarray(inputs["m" + tag], np.float64)
    v = np.asarray(inputs["v" + tag], np.float64)
    wb = np.sign(w)
    s = g / np.sqrt(v + EPS)
    geo = LAYERS[l]
    co, W, parity = geo["co"], geo["W"], geo["parity"]
    halved = dom_c == "tilde"
    wmat = wb * 0.5 if halved else wb
    c = (b - m) + be / s
    flip = np.where(s < 0, -1.0, 1.0)
    wmat = wmat * flip[:, None, None, None]
    taps, colsum = _toeplitz_real(wmat, W, parity)
    kap_col = colsum if halved else np.zeros_like(colsum)
    M = MDIM[l]
    cb = np.zeros(M, np.float64)
    for ox in range(W):
        for c_o in range(co):
            cb[_mcol(ox, c_o, W, co, parity)] = (c * flip)[c_o]
    return taps, kap_col + cb


def _fold_head(inputs, dom_c):
    w5 = np.sign(np.asarray(inputs["w5"], np.float64))
    b5 = np.asarray(inputs["b5"], np.float64)
    g5 = np.asarray(inputs["g5"], np.float64)
    be5 = np.asarray(inputs["be5"], np.float64)
    m5 = np.asarray(inputs["m5"], np.float64)
    v5 = np.asarray(inputs["v5"], np.float64)
    s5 = g5 / np.sqrt(v5 + EPS)
    halved = dom_c == "tilde"
    w5f = w5 * 0.5 if halved else w5
    A = np.zeros((9, 56, 16), np.float64)
    for iy in range(9):
        for ix in range(1, 8):
            for ky in range(3):
                if not (0 <= iy - ky <= 6):
                    continue
                for kx in range(3):
                    if not (0 <= ix - kx <= 6):
                        continue
                    for ci in range(8):
                        A[iy, (ix - 1) * 8 + ci, :] += w5f[:, ci, ky, kx]
    kapA = A.sum(axis=(0, 1)) if halved else np.zeros(16)
    s5h = (s5 / 49.0).astype(np.float32)
    b5h = (s5 * (kapA / 49.0 + b5 - m5) + be5).astype(np.float32)
    return A, s5h, b5h


# fp8 blob layout: conv DR packs per (layer, chunk) + A-matrix packs +
# parity-sum matrices.  DoubleRow ldweights requires the tap-pair stride
# to be a multiple of 16 elements, so packs use Mpad = roundup(M, 16):
# entry -> (col offset, Mpad, M).
def _r16(m):
    return (m + 15) // 16 * 16


_OFF8 = {}
_o = 0


def _add8(key, m):
    global _o
    _OFF8[key] = (_o, _r16(m), m)
    _o += 2 * _r16(m)


for _l in (1, 2, 3, 4):
    for _c in (0, 1):
        _add8((f"{_l}c{_c}", "a"), MDIM[_l])
        _add8((f"{_l}c{_c}", "b"), MDIM[_l])
for _c in (0, 1):
    for _p in range(5):
        _add8((f"Ac{_c}", f"p{_p}"), 16)
for _l in (2, 4):
    _add8((f"P{_l}", "a"), 56)
NB8 = _o
_OFFB = {"hd": (0, 10)}
NBB = 10


def _host_fold(inputs):
    d = {}
    dom = _domains()

    def drpack(taps, mpad):
        K, M = taps.shape[1], taps.shape[2]
        pa = np.zeros((K, 2, mpad))
        pa[:, 0, :M], pa[:, 1, :M] = taps[0], taps[1]
        pb = np.zeros((K, 2, mpad))
        pb[:, 0, :M] = taps[2]
        return pa.reshape(K, -1), pb.reshape(K, -1)

    wf8 = np.zeros((128, NB8), np.float64)
    wbf = np.zeros((128, NBB), np.float64)
    thr = np.zeros((128, 26), np.float32)

    def put8(key, pack, arr):
        off, mpad, _m = _OFF8[(key, pack)]
        wf8[0:arr.shape[0], off:off + 2 * mpad] = arr

    def thrcol(i, vec, engine):
        v_ = -vec if engine == "dve" else vec
        thr[:len(v_), i] = v_.astype(np.float32)

    for l, base in ((1, 0), (2, 2), (3, 4), (4, 6)):
        for c in (0, 1):
            taps, t_ = _fold_layer(inputs, l, dom[(l, c)])
            thrcol(base + c, t_, EVC[(l, c)])
            mpad = _OFF8[(f"{l}c{c}", "a")][1]
            pa, pb = drpack(taps, mpad)
            put8(f"{l}c{c}", "a", pa)
            put8(f"{l}c{c}", "b", pb)

    for c in (0, 1):
        A, s5h, b5h = _fold_head(inputs, dom[(5, c)])
        At = np.concatenate([A, np.zeros((1, 56, 16))], axis=0)  # 10 taps
        for p in range(5):
            put8(f"Ac{c}", f"p{p}",
                 At[2 * p:2 * p + 2].transpose(1, 0, 2).reshape(56, 32))
        # Mpad == M == 16 for A packs, no padding needed
        thr[:16, 9 + c] = b5h
        if c == 0:
            thr[:16, 8] = s5h

    # parity-sum matrices [M_l, 2, 64pad]: evens at 0:56, odds at 64:120
    for l in (2, 4):
        mpad = _OFF8[(f"P{l}", "a")][1]
        P = np.zeros((MDIM[l], 2, mpad), np.float64)
        for m in range(56):
            P[m, :, m] = 1.0
            P[64 + m, :, m] = 1.0
        put8(f"P{l}", "a", P.reshape(MDIM[l], 2 * mpad))

    wl = np.sign(np.asarray(inputs["wl"], np.float64))
    bl = np.asarray(inputs["bl"], np.float64)
    off, width = _OFFB["hd"]
    wbf[0:16, off:off + width] = wl.T * 0.5
    thr[:10, 11] = (bl + 0.5 * wl.sum(axis=1) - CSHIFT).astype(np.float32)
    thr[:10, 12:22] = np.eye(10, dtype=np.float32)
    # evac2 thresholds: tilde bits -> s > -3.5, {0,1} bits -> s > 0.5;
    # ACT Sign uses +bias, DVE is_gt uses the raw threshold
    for i, (l, c) in enumerate(((2, 0), (2, 1), (4, 0), (4, 1))):
        tin = -3.5 if EVC[(l, c)] == "act" else 0.5
        thr[:56, 22 + i] = -tin if EV2[(l, c)] == "act" else tin

    d["wf8"] = wf8.astype(FP8)
    d["wbf"] = wbf.astype(BF16)
    d["thrblob"] = thr
    d["_padv"] = {(l, c): (-1.0 if dom[(l + 1, c)] == "tilde" else 0.0)
                  for l in (1, 2, 3, 4) for c in (0, 1)}
    return d


_CACHE = {}
KREPS = int(os.environ.get("KREPS", "1"))


def _build(padv=None):
    from concourse import bacc, tile, mybir

    dom = _domains()
    if padv is None:
        padv = {(l, c): (-1.0 if dom[(l + 1, c)] == "tilde" else 0.0)
                for l in (1, 2, 3, 4) for c in (0, 1)}

    f32 = mybir.dt.float32
    bf16 = mybir.dt.bfloat16
    fp8 = mybir.dt.float8e4
    ACT = mybir.ActivationFunctionType
    ALU = mybir.AluOpType
    AX = mybir.AxisListType
    DR = mybir.MatmulPerfMode.DoubleRow

    nc = bacc.Bacc("TRN2", num_devices=NCORE)

    xT = {c: nc.dram_tensor(f"xT{c}", (28, SLOTS["xb"] * CH), fp8,
                            kind="ExternalInput") for c in (0, 1)}
    wf8d = nc.dram_tensor("wf8", (128, NB8), fp8, kind="ExternalInput")
    wbfd = nc.dram_tensor("wbf", (128, NBB), bf16, kind="ExternalInput")
    thrd = nc.dram_tensor("thrblob", (128, 26), f32, kind="ExternalInput")
    out = nc.dram_tensor("out", (NPER, 10), f32, kind="ExternalOutput")

    with tile.TileContext(nc) as tc:
        stat = tc.alloc_tile_pool(name="stat", bufs=1)
        scr = tc.alloc_tile_pool(name="scr", bufs=3)
        tl = tc.alloc_tile_pool(name="tl", bufs=8)
        ps = tc.alloc_tile_pool(name="ps", bufs=4, space="PSUM")

        def cpair(name, p, slots, dt):
            return {c: stat.tile([p, slots * CH], dt, tag=f"{name}{c}",
                                 name=f"{name}{c}")
                    for c in (0, 1)}

        xb = cpair("xb", 28, SLOTS["xb"], fp8)
        a2 = cpair("a2", 112, SLOTS["a2"], fp8)
        a3 = cpair("a3", 56, SLOTS["a3"], fp8)
        a4 = cpair("a4", 112, SLOTS["a4"], fp8)
        a5 = cpair("a5", 56, SLOTS["a5"], fp8)

        wf8 = stat.tile([128, NB8], fp8, tag="wf8")
        wbf = stat.tile([128, NBB], bf16, tag="wbf")
        thr = stat.tile([128, 26], f32, tag="thr")
        pos = {0: 0, 1: 0}

        def xdma(i):
            c = i % 2
            s0 = pos[c]
            w_ = XDMA_SPLIT[i]
            nc.sync.dma_start(xb[c][:, s0 * CH:(s0 + w_) * CH],
                              xT[c].ap()[:, s0 * CH:(s0 + w_) * CH])
            pos[c] += w_

        # first x slices + conv weights first: they gate L1 pair 0
        xdma(0)
        nc.sync.dma_start(wf8[:, :], wf8d.ap())
        xdma(1)
        nc.sync.dma_start(thr[:, :], thrd.ap())
        for i in range(2, len(XDMA_SPLIT)):
            xdma(i)
        nc.sync.dma_start(wbf[:, :], wbfd.ap())
        for buf, np_, slots, l in ((a2, 112, (0, 29, 30), 1),
                                   (a3, 56, (0, 15, 16), 2),
                                   (a4, 112, (0, 15, 16), 3),
                                   (a5, 56, (0, 8, 9), 4)):
            for c in (0, 1):
                for s_ in slots:
                    nc.gpsimd.memset(buf[c][0:np_, s_ * CH:(s_ + 1) * CH],
                                     padv[(l, c)])

        def w8v(key, K):
            off, mpad, m = _OFF8[key]
            return wf8[0:K, off:off + 2 * mpad].rearrange(
                "k (h m) -> k h m", h=2)[0:K, :, 0:m]

        whd = wbf[0:16, _OFFB["hd"][0]:_OFFB["hd"][0] + 10]

        def rv(buf, name):
            return {c: buf[c][:, :].rearrange("k (r t) -> k r t",
                                              r=SLOTS[name]) for c in (0, 1)}

        xbv, a2v, a3v, a4v, a5v = (rv(xb, "xb"), rv(a2, "a2"), rv(a3, "a3"),
                                   rv(a4, "a4"), rv(a5, "a5"))

        def conv_chunk(l, srcv, k, c_, pt):
            """DR matmuls for rows (2k, 2k+1) of chunk c_ -> pt halves."""
            K, M = KDIM[l], MDIM[l]
            for h in (0, 1):
                y = 2 * k + h
                q = pt[0:M, h * CH:(h + 1) * CH]
                key = f"{l}c{c_}"
                nc.tensor.matmul(
                    out=q, lhsT=w8v((key, "a"), K),
                    rhs=srcv[c_][0:K, y:y + 2, :],
                    start=True, stop=False, perf_mode=DR, tile_position=(0, 0))
                nc.tensor.matmul(
                    out=q, lhsT=w8v((key, "b"), K),
                    rhs=srcv[c_][0:K, y + 2:y + 4, :],
                    start=False, stop=True, perf_mode=DR, tile_position=(0, 0))

        def evac_chunk(l, pt, c_, dst):
            """evac1: threshold pt [M, 2*CH] into fp8 dst."""
            M = MDIM[l]
            src = pt[0:M, 0:2 * CH]
            tcol = thr[0:M, 2 * (l - 1) + c_:2 * (l - 1) + c_ + 1]
            if EVC[(l, c_)] == "act":
                nc.scalar.activation(dst, src, ACT.Sign, bias=tcol)
            else:
                nc.vector.tensor_scalar(out=dst, in0=src,
                                        scalar1=tcol, scalar2=None,
                                        op0=ALU.is_gt)

        def dst_rows(buf, M, k):
            base = (2 * k + 1) * CH
            return [buf[c_][0:M, base:base + 2 * CH] for c_ in (0, 1)]

        def pool_bits(l, pt, c_):
            """evac1 the pre-pool bits into a held q0 tile."""
            M = MDIM[l]
            q0 = scr.tile([128, 2 * CH], fp8, tag=f"q0{c_}", name=f"q0{c_}",
                          bufs=4)
            evac_chunk(l, pt, c_, q0[0:M, :])
            return q0

        def pool_sum(l, batch, c_, dst):
            """Lagged bit-count matmuls for 1-2 pool rows + one evac2."""
            M = MDIM[l]
            sv = ps.tile([128, 1024], f32, tag="pt", name="sv")
            for j, (q0, _k) in enumerate(batch):
                nc.tensor.matmul(
                    out=sv[0:56, j * CH:(j + 1) * CH],
                    lhsT=w8v((f"P{l}", "a"), M),
                    rhs=q0[0:M, :].rearrange("m (h t) -> m h t", h=2),
                    start=True, stop=True, perf_mode=DR, tile_position=(0, 0))
            # s in {-4..4} (tilde bits) or {0..4} ({0,1} bits); pooled-OR
            ti = {(2, 0): 22, (2, 1): 23, (4, 0): 24, (4, 1): 25}[(l, c_)]
            tcol = thr[0:56, ti:ti + 1]
            n = len(batch)
            k0 = batch[0][1]
            dreg = dst[c_][0:56, (k0 + 1) * CH:(k0 + 1 + n) * CH]
            if EV2[(l, c_)] == "act":
                nc.scalar.activation(dreg, sv[0:56, 0:n * CH], ACT.Sign,
                                     bias=tcol)
            else:
                nc.vector.tensor_scalar(out=dreg, in0=sv[0:56, 0:n * CH],
                                        scalar1=tcol, scalar2=None,
                                        op0=ALU.is_gt)

        srcs = {1: xbv, 2: a2v, 3: a3v, 4: a4v}
        dsts = {2: a3, 4: a5}

        def emit(l, k, pend):
            for c_ in (0, 1):
                pt = ps.tile([128, 1024], f32, tag="pt")
                conv_chunk(l, srcs[l], k, c_, pt)
                if l == 1:
                    evac_chunk(1, pt, c_, dst_rows(a2, 112, k)[c_])
                elif l == 3:
                    evac_chunk(3, pt, c_, dst_rows(a4, 112, k)[c_])
                else:
                    q0 = pool_bits(l, pt, c_)
                    pend[c_].append((q0, k))
                    if len(pend[c_]) == 3:
                        pool_sum(l, pend[c_][:2], c_, dsts[l])
                        pend[c_] = pend[c_][2:]

        def flush(l, pend):
            for c_ in (0, 1):
                while pend[c_]:
                    pool_sum(l, pend[c_][:2], c_, dsts[l])
                    pend[c_] = pend[c_][2:]

        for rep in range(KREPS):
            pend = {0: [], 1: []}
            for k in range(14):
                emit(1, k, pend)
            for k in range(14):
                emit(2, k, pend)
            flush(2, pend)
            for k in range(7):
                emit(3, k, pend)
            for k in range(7):
                emit(4, k, pend)
            flush(4, pend)

            # ---- head: fused conv5 + mean, 10 taps as 5 DR packs ----
            u = scr.tile([16, TP], bf16, tag="u")
            hh = scr.tile([16, TP], f32, tag="hh")
            for c_ in (0, 1):
                pm = ps.tile([128, 1024], f32, tag="pt", name="pm")
                # pack 3 reads the last pool row (slot 7): accumulate it last
                for i_, p_ in enumerate((0, 1, 2, 4, 3)):
                    nc.tensor.matmul(
                        out=pm[0:16, 0:CH],
                        lhsT=w8v((f"Ac{c_}", f"p{p_}"), 56),
                        rhs=a5v[c_][0:56, 2 * p_:2 * p_ + 2, :],
                        start=(i_ == 0), stop=(i_ == 4),
                        perf_mode=DR, tile_position=(0, 0))
                nc.scalar.activation(u[0:16, c_ * CH:(c_ + 1) * CH],
                                     pm[0:16, 0:CH],
                                     ACT.Sign, bias=thr[0:16, 9 + c_:10 + c_],
                                     scale=thr[0:16, 8:9])
            for c_ in (0, 1):
                ph = ps.tile([128, 1024], f32, tag="pt", name="ph")
                nc.tensor.matmul(out=ph[0:10, 0:CH],
                                 lhsT=whd, rhs=u[0:16, c_ * CH:(c_ + 1) * CH],
                                 start=True, stop=True, tile_position=(0, 0))
                nc.scalar.activation(hh[0:10, c_ * CH:(c_ + 1) * CH],
                                     ph[0:10, 0:CH], ACT.Identity,
                                     bias=thr[0:10, 11:12])

            # ---- log-softmax tail ----
            hxs, exs = [], []
            for k in range(8):
                ptr = ps.tile([128, 1024], f32, tag="pt", name="ptr")
                nc.tensor.transpose(ptr[0:128, 0:10],
                                    hh[0:10, k * 128:(k + 1) * 128],
                                    thr[0:10, 12:22])
                hx = tl.tile([128, 16], f32, tag="hx")
                nc.vector.tensor_copy(hx[0:128, 0:10], ptr[0:128, 0:10])
                hxs.append(hx)
                ex = tl.tile([128, 16], f32, tag="ex")
                nc.scalar.activation(ex[0:128, 0:10], ptr[0:128, 0:10],
                                     ACT.Exp)
                exs.append(ex)
            sm = scr.tile([128, 8], f32, tag="sm")
            for k in range(8):
                nc.vector.tensor_reduce(sm[0:128, k:k + 1], exs[k][0:128, 0:10],
                                        axis=AX.X, op=ALU.add)
            lg = scr.tile([128, 8], f32, tag="lg")
            nc.scalar.activation(lg[0:128, 0:8], sm[0:128, 0:8], ACT.Ln)
            osb = scr.tile([128, 80], f32, tag="osb")
            for k in range(8):
                nc.vector.tensor_scalar(
                    out=osb[0:128, k * 10:(k + 1) * 10],
                    in0=hxs[k][0:128, 0:10], scalar1=lg[0:128, k:k + 1],
                    scalar2=None, op0=ALU.subtract)
            outv = out.ap().rearrange("(k r) c -> r k c", k=8)
            nc.sync.dma_start(outv[0:128, 0:4, :], osb[0:128, 0:40])
            nc.sync.dma_start(outv[0:128, 4:8, :], osb[0:128, 40:80])

        for p in (ps, tl, scr, stat):
            p.release()

    nc.compile()
    return nc


def _prep_x(x):
    xs = np.sign(x.reshape(8192, 28, 28).astype(np.float32))
    arr = xs.transpose(2, 1, 0)
    blobs = []
    for co in range(NCORE):
        pair = []
        for c in (0, 1):
            b = np.zeros((28, SLOTS["xb"], CH), np.float32)
            i0 = co * TP + c * CH
            b[:, 1:29, :] = arr[:, :, i0:i0 + CH]
            pair.append(b.reshape(28, SLOTS["xb"] * CH).astype(FP8))
        blobs.append(pair)
    return blobs


def kernel(**inputs):
    from concourse.bass_utils import run_bass_kernel_spmd

    if "nc" not in _CACHE:
        _CACHE["nc"] = _build()
    nc = _CACHE["nc"]

    folded = _host_fold(inputs)
    xblobs = _prep_x(np.asarray(inputs["x"], np.float32))
    static = {k: folded[k] for k in ("wf8", "wbf", "thrblob")}

    in_maps = []
    for i in range(NCORE):
        m = {"xT0": xblobs[i][0], "xT1": xblobs[i][1]}
        m.update(static)
        in_maps.append(m)

    res = run_bass_kernel_spmd(nc, in_maps, core_ids=list(range(NCORE)))
    _CACHE["last_result"] = res
    outs = [res.results[i]["out"] for i in range(NCORE)]
    return np.concatenate(outs, axis=0).astype(np.float32)
